# revision 8
# baseline (speedup 1.0000x reference)
"""Trainium2 Bass kernel for HGBCN message passing (gnn_message_passing).

Computes, for a bipartite user-item hypergraph in COO form:
    node_msg = segment_sum(vals * item_emb[cols], rows)          [n_users, d]
    msg      = Linear(concat([node_msg, node_msg * user_emb]))   [n_users, d]
    norm_emb = segment_sum(vals * msg[rows], cols)               [n_items, d]

Strategy (8 NeuronCores, SPMD, two launches):
  Run A: users row-sharded across cores.  Per core, the incident edges are
  packed (on host) into 128-edge blocks grouped by 128-user destination
  ranges.  Item rows are fetched per edge with dma_gather (bf16, 256B rows)
  and the segment-sum scatter is realized as a matmul with a streamed
  one-hot "S" matrix (S[e, u] = val[e] * (row_local[e] == u)), accumulated
  in PSUM per user range.  The Linear layer is fused in (3 matmuls per
  range: W1 on node_msg, W2 on the gated product, rank-1 bias).
  Run B: items row-sharded.  Same machinery transposed: gather msg rows
  (bf16) per edge, scatter into 128-item ranges via one-hot matmul.
  The COO is deterministic, so the block schedule is baked at trace time;
  it is padded to the max across cores so all 8 cores run one SPMD program.
"""

import sys

if "/opt/trn_rl_repo" not in sys.path:
    sys.path.insert(0, "/opt/trn_rl_repo")

from contextlib import ExitStack

import ml_dtypes
import numpy as np

import concourse.bacc as bacc
import concourse.tile as tile
from concourse import mybir
from concourse._compat import cdiv
from concourse.bass_utils import run_bass_kernel_spmd
from concourse.library_config import mlp

BF16 = ml_dtypes.bfloat16
NCORES = 8
RW = 128         # scatter range width (PSUM tile columns)
MAX_PIECE = 32768  # int16 gather index limit per source table piece
GCH = 64         # gather chunk, in 128-edge blocks (8192 indices per call;
                 # needs single_packet=False, HW rejects >8192 per call)
SCH = 16         # S-stream DMA chunk, in blocks


def _wrap_idx(idx):
    """int16 index stream -> [128, n/16] SBUF layout (wrapped in 16
    partitions, replicated 8x across the 128 partitions)."""
    n = len(idx)
    ns = cdiv(n, 16)
    pad = np.zeros(ns * 16, np.int16)
    pad[:n] = idx
    w16 = pad.reshape(ns, 16).T
    return np.ascontiguousarray(np.tile(w16, (8, 1)))


def _pack(dst, src, vals, dpc, n_src, ncores):
    """Pack COO edges into per-core, per-source-piece block streams.

    dst: scatter destination per edge (cores own contiguous dpc-sized slabs)
    src: gather source row per edge (split into <=MAX_PIECE row pieces so
         indices fit int16)
    Returns (B, Btot, NP, psz, npieces, percore) where
      B[r, p]   = blocks for range r from piece p (max over cores, SPMD pad)
      Btot      = total blocks per core
      NP[p]     = index-stream length for piece p
      percore   = list of (idx_streams[p] wrapped, S3 [128, Btot, 128] bf16)
    """
    n_ranges = cdiv(dpc, RW)
    npieces = cdiv(n_src, MAX_PIECE)
    psz = cdiv(n_src, npieces)
    dst = dst.astype(np.int64)
    src = src.astype(np.int64)
    core_id = dst // dpc
    E = []
    cnts = np.zeros((ncores, n_ranges * npieces), np.int64)
    for c in range(ncores):
        m = core_id == c
        d_l = dst[m] - c * dpc
        s = src[m]
        v = vals[m]
        p_e = s // psz
        s_l = (s - p_e * psz).astype(np.int16)
        r_e = d_l // RW
        grp = r_e * npieces + p_e
        order = np.argsort(grp, kind="stable")
        cnt = np.bincount(grp, minlength=n_ranges * npieces)
        cnts[c] = cnt
        E.append((grp, order, cnt, s_l, v, d_l, p_e, r_e))
    B = -(-cnts.max(0).reshape(n_ranges, npieces) // 128)  # ceil
    # every range needs >= 1 block so its PSUM group exists
    empty = B.sum(1) == 0
    B[empty, 0] = 1
    Bp_cum = np.vstack([np.zeros((1, npieces), np.int64), np.cumsum(B, 0)])
    Brow = B.sum(1)
    blkbase = np.concatenate([[0], np.cumsum(Brow)])
    piece_off = np.hstack([np.zeros((n_ranges, 1), np.int64), np.cumsum(B, 1)])
    Btot = int(Brow.sum())
    NP = B.sum(0) * 128
    percore = []
    for c in range(ncores):
        grp, order, cnt, s_l, v, d_l, p_e, r_e = E[c]
        start = np.concatenate([[0], np.cumsum(cnt)])
        k = np.empty(len(grp), np.int64)
        k[order] = np.arange(len(grp)) - start[grp[order]]
        slot = Bp_cum[r_e, p_e] * 128 + k
        gblk = blkbase[r_e] + piece_off[r_e, p_e] + k // 128
        eslot = k % 128
        doff = d_l % RW
        idx_streams = []
        for p in range(npieces):
            st = np.zeros(int(NP[p]), np.int16)
            mm = p_e == p
            st[slot[mm]] = s_l[mm]
            idx_streams.append(_wrap_idx(st))
        S3 = np.zeros((128, Btot, 128), BF16)
        S3[eslot, gblk, doff] = v.astype(BF16)
        percore.append((idx_streams, S3))
    return B, Btot, NP, psz, npieces, percore


def _emit_block_loop(nc, tc, ctx, B, Btot, NPs, tabs, idx_dr, s_dr, stationary_is_s):
    """Shared block-stream machinery: gather chunks + S chunks + per-range
    PSUM-accumulated one-hot matmuls.  Yields (r, psum_tile) per range."""
    dt = mybir.dt
    n_ranges, npieces = B.shape
    # keep all gather pools within ~64KB/partition: npieces * 2 bufs * gch * 256B
    gch = min(GCH, max(8, (64 * 1024) // (npieces * 2 * 256)))
    consts = ctx.enter_context(tc.tile_pool(name="idxc", bufs=1))
    gpools = [
        ctx.enter_context(tc.tile_pool(name=f"g{p}", bufs=2)) for p in range(npieces)
    ]
    spool = ctx.enter_context(tc.tile_pool(name="spool", bufs=4))
    pp = ctx.enter_context(tc.tile_pool(name="pp", bufs=4, space="PSUM"))

    idx_t = []
    for p in range(npieces):
        t = consts.tile([128, cdiv(int(NPs[p]), 16)], dt.int16, tag=f"idx{p}")
        nc.sync.dma_start(t[:], idx_dr[p].ap())
        idx_t.append(t)

    nblk_p = [int(B[:, p].sum()) for p in range(npieces)]
    g_chunks = [[] for _ in range(npieces)]
    s_chunks = []

    def ensure_g(p, blk):
        ch = blk // gch
        while len(g_chunks[p]) <= ch:
            i = len(g_chunks[p])
            nb = min(gch, nblk_p[p] - i * gch)
            t = gpools[p].tile([128, gch, 128], dt.bfloat16, tag=f"g{p}")
            nidx = nb * 128
            nc.gpsimd.dma_gather(
                t[:, :nb, :],
                tabs[p].ap(),
                idx_t[p][:, i * (gch * 8) : i * (gch * 8) + cdiv(nidx, 16)],
                nidx,
                nidx,
                128,
                single_packet=False,
            )
            g_chunks[p].append(t)
        return g_chunks[p][ch]

    def ensure_s(gb):
        ch = gb // SCH
        while len(s_chunks) <= ch:
            i = len(s_chunks)
            nb = min(SCH, Btot - i * SCH)
            t = spool.tile([128, SCH, 128], dt.bfloat16, tag="s")
            nc.sync.dma_start(t[:, :nb, :], s_dr.ap()[:, i * SCH : i * SCH + nb, :])
            s_chunks.append(t)
        return s_chunks[ch]

    pblk = [0] * npieces
    gblk = 0
    for r in range(n_ranges):
        nb_r = int(B[r].sum())
        ps = pp.tile([128, 128], dt.float32, tag="ps")
        j = 0
        for p in range(npieces):
            for _ in range(int(B[r, p])):
                gt = ensure_g(p, pblk[p])
                st = ensure_s(gblk)
                g_ap = gt[:, pblk[p] % gch, :]
                s_ap = st[:, gblk % SCH, :]
                if stationary_is_s:
                    nc.tensor.matmul(
                        ps[:], s_ap, g_ap, start=(j == 0), stop=(j == nb_r - 1)
                    )
                else:
                    nc.tensor.matmul(
                        ps[:], g_ap, s_ap, start=(j == 0), stop=(j == nb_r - 1)
                    )
                pblk[p] += 1
                gblk += 1
                j += 1
        yield r, ps


def _build_a(B, Btot, NPs, upc, tab_sizes):
    """Run A: node_msg (dim-major PSUM) + fused Linear -> msg rows (f32)."""
    dt = mybir.dt
    n_ranges, npieces = B.shape
    nc = bacc.Bacc("TRN2", target_bir_lowering=False, debug=False, num_devices=NCORES)
    tabs = [
        nc.dram_tensor(f"tab{p}", [sz, 128], dt.bfloat16, kind="ExternalInput")
        for p, sz in enumerate(tab_sizes)
    ]
    idx_dr = [
        nc.dram_tensor(
            f"idx{p}", [128, cdiv(int(NPs[p]), 16)], dt.int16, kind="ExternalInput"
        )
        for p in range(npieces)
    ]
    s_dr = nc.dram_tensor("smat", [128, Btot, 128], dt.bfloat16, kind="ExternalInput")
    uembT = nc.dram_tensor("uembT", [128, upc], dt.bfloat16, kind="ExternalInput")
    w1t = nc.dram_tensor("w1t", [128, 128], dt.bfloat16, kind="ExternalInput")
    w2t = nc.dram_tensor("w2t", [128, 128], dt.bfloat16, kind="ExternalInput")
    bias = nc.dram_tensor("bias", [1, 128], dt.bfloat16, kind="ExternalInput")
    msg_o = nc.dram_tensor("msg", [upc, 128], dt.float32, kind="ExternalOutput")

    with tile.TileContext(nc) as tc, ExitStack() as ctx:
        nc.gpsimd.load_library(mlp)
        cpool = ctx.enter_context(tc.tile_pool(name="consts", bufs=1))
        nmpool = ctx.enter_context(tc.tile_pool(name="nm", bufs=1))
        gatedp = ctx.enter_context(tc.tile_pool(name="gated", bufs=2))
        msgp = ctx.enter_context(tc.tile_pool(name="msgout", bufs=3))
        pp2 = ctx.enter_context(tc.tile_pool(name="pp2", bufs=2, space="PSUM"))

        uembT_t = cpool.tile([128, upc], dt.bfloat16)
        nc.sync.dma_start(uembT_t[:], uembT.ap())
        w1_t = cpool.tile([128, 128], dt.bfloat16)
        nc.sync.dma_start(w1_t[:], w1t.ap())
        w2_t = cpool.tile([128, 128], dt.bfloat16)
        nc.sync.dma_start(w2_t[:], w2t.ap())
        b_t = cpool.tile([1, 128], dt.bfloat16)
        nc.sync.dma_start(b_t[:], bias.ap())
        ones_t = cpool.tile([1, 128], dt.bfloat16)
        nc.vector.memset(ones_t[:], 1.0)
        nmT = nmpool.tile([128, upc], dt.bfloat16)

        for r, ps in _emit_block_loop(
            nc, tc, ctx, B, Btot, NPs, tabs, idx_dr, s_dr, stationary_is_s=False
        ):
            u0 = r * RW
            wu = min(RW, upc - u0)
            nc.vector.tensor_copy(nmT[:, u0 : u0 + wu], ps[:, :wu])
            gt2 = gatedp.tile([128, 128], dt.bfloat16, tag="gated")
            nc.vector.tensor_mul(
                gt2[:, :wu], nmT[:, u0 : u0 + wu], uembT_t[:, u0 : u0 + wu]
            )
            pm = pp2.tile([128, 128], dt.float32, tag="pm")
            nc.tensor.matmul(
                pm[:wu, :], nmT[:, u0 : u0 + wu], w1_t[:], start=True, stop=False
            )
            nc.tensor.matmul(pm[:wu, :], gt2[:, :wu], w2_t[:], start=False, stop=False)
            nc.tensor.matmul(
                pm[:wu, :], ones_t[:, :wu], b_t[:], start=False, stop=True
            )
            mo = msgp.tile([128, 128], dt.float32, tag="mo")
            nc.vector.tensor_copy(mo[:wu, :], pm[:wu, :])
            nc.sync.dma_start(msg_o.ap()[u0 : u0 + wu, :], mo[:wu, :])
    nc.compile()
    return nc


def _build_b(B, Btot, NPs, ipc, tab_sizes):
    """Run B: norm_emb rows (f32) via one-hot scatter of gathered msg rows."""
    dt = mybir.dt
    n_ranges, npieces = B.shape
    nc = bacc.Bacc("TRN2", target_bir_lowering=False, debug=False, num_devices=NCORES)
    tabs = [
        nc.dram_tensor(f"tab{p}", [sz, 128], dt.bfloat16, kind="ExternalInput")
        for p, sz in enumerate(tab_sizes)
    ]
    idx_dr = [
        nc.dram_tensor(
            f"idx{p}", [128, cdiv(int(NPs[p]), 16)], dt.int16, kind="ExternalInput"
        )
        for p in range(npieces)
    ]
    s_dr = nc.dram_tensor("smat", [128, Btot, 128], dt.bfloat16, kind="ExternalInput")
    norm_o = nc.dram_tensor("norm", [ipc, 128], dt.float32, kind="ExternalOutput")

    with tile.TileContext(nc) as tc, ExitStack() as ctx:
        nc.gpsimd.load_library(mlp)
        outp = ctx.enter_context(tc.tile_pool(name="outp", bufs=3))
        for r, ps in _emit_block_loop(
            nc, tc, ctx, B, Btot, NPs, tabs, idx_dr, s_dr, stationary_is_s=True
        ):
            i0 = r * RW
            wi = min(RW, ipc - i0)
            no = outp.tile([128, 128], dt.float32, tag="no")
            nc.vector.tensor_copy(no[:wi, :], ps[:wi, :])
            nc.sync.dma_start(norm_o.ap()[i0 : i0 + wi, :], no[:wi, :])
    nc.compile()
    return nc


def _pieces(total, psz):
    return [min(psz, total - p * psz) for p in range(cdiv(total, psz))]


_LAST = {}


def _run(nc, in_maps):
    return run_bass_kernel_spmd(nc, in_maps, core_ids=list(range(NCORES))).results


def _compute(user_emb, item_emb, hg_rows, hg_cols, hg_vals, W_agg, b_agg, runner=_run):
    n_users, dim = user_emb.shape
    n_items = item_emb.shape[0]
    assert dim == 128 and n_users % NCORES == 0 and n_items % NCORES == 0
    upc = n_users // NCORES
    ipc = n_items // NCORES

    # ---------------- Run A: users sharded ----------------
    B1, Btot1, NP1, psz1, npc1, packs1 = _pack(
        hg_rows, hg_cols, hg_vals, upc, n_items, NCORES
    )
    tab1_sizes = _pieces(n_items, psz1)
    ncA = _build_a(B1, Btot1, NP1, upc, tab1_sizes)
    _LAST["ncA"] = ncA
    item_bf = item_emb.astype(BF16)
    item_pieces = [
        np.ascontiguousarray(item_bf[p * psz1 : p * psz1 + sz])
        for p, sz in enumerate(tab1_sizes)
    ]
    w1t = np.ascontiguousarray(W_agg[:, :128].T).astype(BF16)
    w2t = np.ascontiguousarray(W_agg[:, 128:].T).astype(BF16)
    bias = b_agg.reshape(1, 128).astype(BF16)
    in_maps_a = []
    for c in range(NCORES):
        idx_streams, S3 = packs1[c]
        m = {f"tab{p}": item_pieces[p] for p in range(npc1)}
        m |= {f"idx{p}": idx_streams[p] for p in range(npc1)}
        m["smat"] = S3
        m["uembT"] = np.ascontiguousarray(
            user_emb[c * upc : (c + 1) * upc].T.astype(BF16)
        )
        m["w1t"] = w1t
        m["w2t"] = w2t
        m["bias"] = bias
        in_maps_a.append(m)
    resA = runner(ncA, in_maps_a)
    msg = np.concatenate([resA[c]["msg"] for c in range(NCORES)], axis=0)

    # ---------------- Run B: items sharded ----------------
    B3, Btot3, NP3, psz3, npc3, packs3 = _pack(
        hg_cols, hg_rows, hg_vals, ipc, n_users, NCORES
    )
    tab3_sizes = _pieces(n_users, psz3)
    ncB = _build_b(B3, Btot3, NP3, ipc, tab3_sizes)
    _LAST["ncB"] = ncB
    msg_bf = msg.astype(BF16)
    msg_pieces = [
        np.ascontiguousarray(msg_bf[p * psz3 : p * psz3 + sz])
        for p, sz in enumerate(tab3_sizes)
    ]
    in_maps_b = []
    for c in range(NCORES):
        idx_streams, S3 = packs3[c]
        m = {f"tab{p}": msg_pieces[p] for p in range(npc3)}
        m |= {f"idx{p}": idx_streams[p] for p in range(npc3)}
        m["smat"] = S3
        in_maps_b.append(m)
    resB = runner(ncB, in_maps_b)
    norm_emb = np.concatenate([resB[c]["norm"] for c in range(NCORES)], axis=0)
    return norm_emb.astype(np.float32), msg.astype(np.float32)


def kernel(user_emb, item_emb, hg_rows, hg_cols, hg_vals, W_agg, b_agg):
    return _compute(
        np.asarray(user_emb, np.float32),
        np.asarray(item_emb, np.float32),
        np.asarray(hg_rows),
        np.asarray(hg_cols),
        np.asarray(hg_vals, np.float32),
        np.asarray(W_agg, np.float32),
        np.asarray(b_agg, np.float32),
    )
